# revision 37
# baseline (speedup 1.0000x reference)
"""Trainium2 Bass kernel for nn_Attention_88493506167116.

Channel-attention module (per batch item b):
    F = (Wf @ X).reshape raw (N, C);  G likewise;  Hm likewise (X = x[b] as (C, N))
    S = G^T @ F  (C x C), beta = softmax(S, axis=-1)
    O = beta @ Hm^T  (C, N) -> reshape (C, W, H);  out = Wo @ O + bo

Key structure (C=256, N=4096 = 16*C): the raw reshape (C, N) -> (N, C) is a
block regrouping: F_r[16c+q, r] = Yf[c, q*C + r].  Hence with X_q = X[:, qC:(q+1)C]:
    S     = sum_q Yg_q^T Yf_q = sum_q X_q^T A X_q,   A = Wg^T Wf   (host-folded)
    out   = Wo @ O: with P = Wo @ beta, Out[o, 16c+q] = (P @ Yh_q^T)[o, c]
and Yh_q^T = YhT[qC:(q+1)C, :] where YhT = X^T Wh^T is computed directly in
transposed layout (no on-device transposes anywhere).

Sharding: pure data-parallel, batch B=8 across the 8 NeuronCores (one image
per core), zero collectives.  Host folds A = Wf.T @ Wg (lhsT layout) and
transposes weights.

The fast (zero-bias) path computes in float32r: fp32 storage/exponent, PE
rounds products to ~13 mantissa bits, 4x faster than fp32 matmul.  Measured
end-to-end error ~1.3e-3.  The general-bias path stays full fp32.

Hardware constraints honored: fp32/f32r matmuls self-load weights (S3_LW)
and, like DMA instructions, carry at most ONE sync wait.  So: weights packed
into one contiguous DMA; tiny fp32 warmup matmuls cover each DMA sem lane
before first use; PSUM pools are split so each pool's tiles are only ever
read by one engine (psW -> ACT, psV -> DVE), making every matmul's WAR +
input waits coalesce onto a single semaphore.  A post-pass splits any
residual multi-wait instruction (the tail drain) into single-wait no-ops.
"""

import numpy as np

B, C, W_DIM, H_DIM = 8, 256, 64, 64
N = W_DIM * H_DIM          # 4096
Q = N // C                 # 16
P = 128                    # partitions
NCORES = 8

_GRAPH_CACHE = {}


def _build_graph(use_bias: bool):
    from contextlib import ExitStack

    import concourse.bass as bass
    import concourse.tile as tile
    from concourse import mybir

    f32 = mybir.dt.float32
    f32r = mybir.dt.float32r
    AF = mybir.ActivationFunctionType

    nc = bass.Bass()

    dcomp = f32 if use_bias else f32r

    NW = 8 if use_bias else 6
    x_ext = nc.declare_dram_parameter("x", [C, N], dcomp, isOutput=False)
    # wpk is pre-swizzled on host to (P, NW*C): partition-major, contiguous
    # 6KB lines per partition -> fast DMA (vs 768 separate 1KB descriptors)
    wpk_ext = nc.declare_dram_parameter("wpk", [P, NW * C], dcomp, isOutput=False)
    if use_bias:
        bpk_ext = nc.declare_dram_parameter("bpk", [P, 6], f32, isOutput=False)
        bhw_ext = nc.declare_dram_parameter("bhw", [1, 3 * C], f32, isOutput=False)
    dout = f32 if use_bias else mybir.dt.bfloat16
    out_ext = nc.declare_dram_parameter("out", [C, N], dout, isOutput=True)

    with ExitStack() as ctx:
        tc = ctx.enter_context(tile.TileContext(nc))
        cpool = ctx.enter_context(tc.tile_pool(name="cpool", bufs=1))
        psS = ctx.enter_context(tc.tile_pool(name="psS", bufs=1, space="PSUM"))
        psW = ctx.enter_context(tc.tile_pool(name="psW", bufs=3, space="PSUM"))
        psV = ctx.enter_context(tc.tile_pool(name="psV", bufs=3, space="PSUM"))

        # pool-consistent evacuation engines: psW tiles are read only by the
        # scalar engine (ACT), psV tiles only by the vector engine (DVE)
        def evacA(dst, src):
            nc.scalar.copy(dst, src)

        def evacV(dst, src):
            nc.vector.tensor_copy(dst, src)

        if use_bias:
            evacV = evacA  # single engine keeps the wait discipline trivial

        def pick(i):
            """alternate (pool, evac) by index for load balance"""
            if use_bias:
                return psW, evacA
            return (psV, evacV) if i % 2 == 0 else (psW, evacA)

        # ---- resident SBUF tensors -------------------------------------
        # x loaded in column pieces per row-chunk (small first piece) so
        # compute can start as soon as ~0.5 MB lands.
        x_sb = [cpool.tile([P, N], dcomp, name=f"x{kc}") for kc in range(2)]
        wpk_sb = cpool.tile([P, NW, C], dcomp, name="wpk")
        nc.sync.dma_start(out=wpk_sb.rearrange("p a b -> p (a b)"), in_=wpk_ext[:])
        XCUTS = [0, 512, 2048, N]
        for h in range(len(XCUTS) - 1):
            for kc in range(2):
                nc.sync.dma_start(
                    out=x_sb[kc][:, XCUTS[h]:XCUTS[h + 1]],
                    in_=x_ext[kc * P:(kc + 1) * P, XCUTS[h]:XCUTS[h + 1]])

        if use_bias:
            wft_sb = [wpk_sb[:, 0 + kc, :] for kc in range(2)]
            wgt_sb = [wpk_sb[:, 2 + kc, :] for kc in range(2)]
            wht_sb = [wpk_sb[:, 4 + kc, :] for kc in range(2)]
            wot_sb = [wpk_sb[:, 6 + kc, :] for kc in range(2)]
            bpk_sb = cpool.tile([P, 6], f32, name="bpk")
            bhw_sb = cpool.tile([1, 3 * C], f32, name="bhw")
            nc.sync.dma_start(out=bpk_sb[:], in_=bpk_ext[:])
            nc.sync.dma_start(out=bhw_sb[:], in_=bhw_ext[:])
            bf_sb = [bpk_sb[:, 0 + kc:1 + kc] for kc in range(2)]
            bg_sb = [bpk_sb[:, 2 + kc:3 + kc] for kc in range(2)]
            bo_sb = [bpk_sb[:, 4 + kc:5 + kc] for kc in range(2)]
            bh2_row = bhw_sb[0:1, 0:2 * C]       # [bh | bh]
            wosum_row = bhw_sb[0:1, 2 * C:3 * C]
        else:
            at_sb = [wpk_sb[:, 0 + kc, :] for kc in range(2)]
            wht_sb = [wpk_sb[:, 2 + kc, :] for kc in range(2)]
            wot_sb = [wpk_sb[:, 4 + kc, :] for kc in range(2)]

        # ---- PE sync warmup (one tiny fp32 matmul per DMA sem lane) ----
        scratch_ps = psV.tile([P, 512], f32, name="v")

        def warmup(t):
            nc.tensor.matmul(scratch_ps[:, 0:1], t.bitcast(f32),
                             t[:, 0:1].bitcast(f32), start=True, stop=True)

        if not use_bias:
            # HAM pre-warm: keep PE busy on dummy fp32 matmuls while the x /
            # weight DMAs land, so real matmuls start at 2.4 GHz not 1.2.
            # Emitted before the lane warmups (those block on the DMAs).
            # fp32 matmuls lower to two half-speed passes, so each dummy is
            # ~0.4-0.9us; 8 of them spans the DMA head: HAM flips to 8/8
            # after ~3.4us of PE busy and stays warm into the real work
            ham_ct = cpool.tile([P, 256], f32, name="ham_ct")
            nc.vector.memset(ham_ct[:], 0.7182818)
            for _ in range(8):
                nc.tensor.matmul(scratch_ps[:, 0:256], ham_ct[:, 0:P],
                                 ham_ct[:], start=True, stop=True)

        warmup(wpk_sb[:, 0, 0:P])
        warmup(x_sb[0][:, 0:P])
        warmup(x_sb[1][:, 0:P])
        if use_bias:
            for h in range(1, 3):
                warmup(x_sb[0][:, XCUTS[h]:XCUTS[h] + P])
                warmup(x_sb[1][:, XCUTS[h]:XCUTS[h] + P])
            nc.tensor.matmul(scratch_ps[0:1, 0:1], bhw_sb[0:1, 0:1],
                             bhw_sb[0:1, 0:1], start=True, stop=True)
            act_scr = cpool.tile([P, 1], f32, name="act_scr")
            nc.scalar.copy(act_scr[:], bpk_sb[:, 0:1])

        # S accumulator PSUM tiles, pinned across the whole contraction
        psS_t = [psS.tile([P, C], f32, name=f"S{ac}") for ac in range(2)]

        # ================================================================
        # Pre-softmax path: fill psS_t[ac] with S = G_r^T F_r
        # ================================================================
        if not use_bias:
            # T_q = A @ X_q (two q at a time, 512-wide), then S += X_q^T T_q
            t2_sb = [[cpool.tile([P, 2 * C], dcomp, name=f"t{qp}_{uc}")
                      for uc in range(2)] for qp in range(Q // 2)]
            for qp in range(Q // 2):
                if qp in (1, 4):  # next x column piece arrives
                    h = 1 if qp == 1 else 2
                    warmup(x_sb[0][:, XCUTS[h]:XCUTS[h] + P])
                    warmup(x_sb[1][:, XCUTS[h]:XCUTS[h] + P])
                for uc in range(2):
                    pool, ev = pick(qp * 2 + uc)
                    ps = pool.tile([P, 2 * C], f32,
                                   name="v" if pool is psV else "w")
                    for kc in range(2):
                        nc.tensor.matmul(
                            ps[:],
                            at_sb[kc][:, uc * P:(uc + 1) * P],
                            x_sb[kc][:, qp * 2 * C:(qp + 1) * 2 * C],
                            start=(kc == 0), stop=(kc == 1),
                        )
                    ev(t2_sb[qp][uc][:], ps[:])
            for ac in range(2):
                for q in range(Q):
                    for uc in range(2):
                        nc.tensor.matmul(
                            psS_t[ac][:],
                            x_sb[uc][:, q * C + ac * P: q * C + ac * P + P],
                            t2_sb[q // 2][uc][:, (q % 2) * C:(q % 2 + 1) * C],
                            start=(q == 0 and uc == 0),
                            stop=(q == Q - 1 and uc == 1),
                        )
        else:
            # materialize Yf = Wf X + bf and Yg = Wg X + bg, then
            # S = sum_q Yg_q^T Yf_q
            yf_sb = [cpool.tile([P, N], f32, name=f"yf{mc}") for mc in range(2)]
            yg_sb = [cpool.tile([P, N], f32, name=f"yg{mc}") for mc in range(2)]
            for mc in range(2):
                for nb in range(8):
                    nsl = slice(nb * 512, (nb + 1) * 512)
                    ps = psW.tile([P, 512], f32, name="w")
                    for kc in range(2):
                        nc.tensor.matmul(
                            ps[:], wft_sb[kc][:, mc * P:(mc + 1) * P],
                            x_sb[kc][:, nsl], start=(kc == 0), stop=(kc == 1))
                    nc.scalar.activation(yf_sb[mc][:, nsl], ps[:], AF.Identity,
                                         bias=bf_sb[mc], scale=1.0)
                    ps = psW.tile([P, 512], f32, name="w")
                    for kc in range(2):
                        nc.tensor.matmul(
                            ps[:], wgt_sb[kc][:, mc * P:(mc + 1) * P],
                            x_sb[kc][:, nsl], start=(kc == 0), stop=(kc == 1))
                    nc.scalar.activation(yg_sb[mc][:, nsl], ps[:], AF.Identity,
                                         bias=bg_sb[mc], scale=1.0)
            for ac in range(2):
                for q in range(Q):
                    for kc in range(2):
                        nc.tensor.matmul(
                            psS_t[ac][:],
                            yg_sb[kc][:, q * C + ac * P: q * C + ac * P + P],
                            yf_sb[kc][:, q * C:(q + 1) * C],
                            start=(q == 0 and kc == 0),
                            stop=(q == Q - 1 and kc == 1),
                        )

        # ================================================================
        # YhT = X^T @ Wh^T in (N, C) layout: 8 quad tiles (128, 4C), quad u
        # holds row-chunks 4u..4u+3 at column offsets j*C
        # ================================================================
        yht_q4 = [cpool.tile([P, 4 * C], dcomp, name=f"yht{u}")
                  for u in range(Q // 2)]
        for u in range(Q // 2):
            for g in range(2):
                pool, ev = pick(2 * u + g)
                ps = pool.tile([P, 2 * C], f32, name="v" if pool is psV else "w")
                for half in range(2):
                    nch = 4 * u + 2 * g + half
                    for kc in range(2):
                        nc.tensor.matmul(
                            ps[:, half * C:(half + 1) * C],
                            x_sb[kc][:, nch * P:(nch + 1) * P],
                            wht_sb[kc][:],
                            start=(kc == 0), stop=(kc == 1),
                        )
                ev(yht_q4[u][:, g * 2 * C:(g + 1) * 2 * C], ps[:])

        # ================================================================
        # softmax rows of S -> beta (normalized), in SBUF
        # ================================================================
        beta_sb = [cpool.tile([P, C], dcomp, name=f"beta{ac}") for ac in range(2)]
        for ac in range(2):
            negmax = cpool.tile([P, 1], f32, name=f"negmax{ac}")
            sumexp = cpool.tile([P, 1], f32, name=f"sumexp{ac}")
            rcp = cpool.tile([P, 1], f32, name=f"rcp{ac}")
            expo = cpool.tile([P, C], f32, name=f"expo{ac}")
            nc.vector.tensor_reduce(
                out=negmax[:], in_=psS_t[ac][:],
                axis=mybir.AxisListType.X, op=mybir.AluOpType.max, negate=True)
            nc.scalar.activation(
                expo[:], psS_t[ac][:], AF.Exp,
                bias=negmax[:, 0:1], scale=1.0, accum_out=sumexp[:, 0:1])
            nc.vector.reciprocal(rcp[:], sumexp[:])
            if use_bias:
                nc.scalar.activation(beta_sb[ac][:], expo[:], AF.Copy,
                                     bias=0.0, scale=rcp[:, 0:1])
            else:
                nc.vector.tensor_scalar_mul(beta_sb[ac][:], expo[:], rcp[:, 0:1])

        # ================================================================
        # P^T = beta^T @ Wo^T   (2 tiles (128, C), j' on partitions)
        # ================================================================
        pt_sb = [cpool.tile([P, C], dcomp, name=f"pt{j}") for j in range(2)]
        for jpc in range(2):
            pool = psW if use_bias else psV
            ps = pool.tile([P, 2 * C], f32, name="w" if use_bias else "v")
            for jc in range(2):
                nc.tensor.matmul(
                    ps[:, 0:C],
                    beta_sb[jc][:, jpc * P:(jpc + 1) * P],
                    wot_sb[jc][:],
                    start=(jc == 0), stop=(jc == 1),
                )
            (evacA if use_bias else evacV)(pt_sb[jpc][:], ps[:, 0:C])

        # ================================================================
        # Out[o, 16c+q] = (P @ Yh_q^T)[o, c] (+ wosum[o]*bh[c] + bo[o])
        # Two q per PSUM tile; one paired (transposing-AP) evacuation.
        # ================================================================
        for oc in range(2):
            out_sb = cpool.tile([P, C, Q], dout, name=f"out{oc}")
            for u in range(Q // 2):
                pool, ev = pick(u + oc)
                ps = pool.tile([P, 2 * C], f32, name="v" if pool is psV else "w")
                # rhs covers q=2u (cols 0:C) and q=2u+1 (C:2C) in one 512-wide
                # strided stream: chunks {4u+jc, 4u+2+jc} of YhT
                rhsv = yht_q4[u].rearrange("p (x y c) -> p y x c", x=2, y=2)
                for jc in range(2):
                    nc.tensor.matmul(
                        ps[:],
                        pt_sb[jc][:, oc * P:(oc + 1) * P],
                        rhsv[:, jc],
                        start=(jc == 0),
                        stop=(jc == 1 and not use_bias),
                    )
                if use_bias:
                    nc.tensor.matmul(
                        ps[:],
                        wosum_row[:, oc * P:(oc + 1) * P],
                        bh2_row[:],
                        start=False, stop=True,
                    )
                    nc.scalar.activation(
                        out_sb[:, :, 2 * u:2 * u + 2],
                        ps.rearrange("p (h c) -> p c h", h=2),
                        AF.Identity, bias=bo_sb[oc], scale=1.0)
                else:
                    ev(out_sb[:, :, 2 * u:2 * u + 2],
                       ps.rearrange("p (h c) -> p c h", h=2))
            nc.sync.dma_start(
                out=out_ext[oc * P:(oc + 1) * P, :],
                in_=out_sb.rearrange("p c q -> p (c q)"),
            )

    return nc


def _split_multiwait_insts(nc, max_waits: int = 1):
    """walrus rejects instructions carrying more than one sync wait; hoist
    extra waits onto same-engine no-ops placed immediately before."""
    from concourse import mybir

    nop_id = 0
    for fn in nc.m.functions:
        for blk in fn.blocks:
            insts = list(blk.instructions)
            new_list = []
            changed = False
            for inst in insts:
                si = inst.sync_info
                if si is not None and len(si.on_wait) > max_waits:
                    waits = list(si.on_wait)
                    for w in waits[:-max_waits]:
                        nop = mybir.InstNoOp(name=f"I-waitnop{nop_id}", ins=[],
                                             outs=[])
                        nop_id += 1
                        nop.engine = inst.engine
                        nop.sync_info = mybir.SyncInfo(on_wait=[w], on_update=[])
                        new_list.append(nop)
                    inst.sync_info = mybir.SyncInfo(
                        on_wait=waits[-max_waits:],
                        on_update=list(si.on_update),
                    )
                    changed = True
                new_list.append(inst)
            if changed:
                blk.instructions = new_list
    return nc


def _get_graph(use_bias: bool):
    key = bool(use_bias)
    if key not in _GRAPH_CACHE:
        _GRAPH_CACHE[key] = _split_multiwait_insts(_build_graph(key))
    return _GRAPH_CACHE[key]


def _make_in_maps(inputs, use_bias):
    x = np.ascontiguousarray(np.asarray(inputs["x"], dtype=np.float32))
    Wf = np.asarray(inputs["Wf"], dtype=np.float32)
    Wg = np.asarray(inputs["Wg"], dtype=np.float32)
    Wh = np.asarray(inputs["Wh"], dtype=np.float32)
    Wo = np.asarray(inputs["Wo"], dtype=np.float32)

    wht = np.ascontiguousarray(Wh.T)
    wot = np.ascontiguousarray(Wo.T)

    def swizzle(wlist):
        # stack (NW, 128, C) row-chunks then move partitions outermost:
        # wpk[p, g*C:(g+1)*C] = chunk g row p  ->  shape (P, NW*C)
        chunks = []
        for w in wlist:
            chunks.append(w[:P])
            chunks.append(w[P:])
        arr = np.stack(chunks, axis=0)           # (NW, P, C)
        return np.ascontiguousarray(
            arr.transpose(1, 0, 2).reshape(P, -1))

    if use_bias:
        bf = np.asarray(inputs["bf"], np.float32)
        bg = np.asarray(inputs["bg"], np.float32)
        bh = np.asarray(inputs["bh"], np.float32)
        bo = np.asarray(inputs["bo"], np.float32)
        wpk = swizzle([Wf.T, Wg.T, wht, wot])
        bpk = np.stack([bf[:P], bf[P:], bg[:P], bg[P:], bo[:P], bo[P:]], axis=1)
        bhw = np.concatenate([bh, bh, Wo.sum(axis=1)]).reshape(1, 3 * C)
        common = {
            "wpk": wpk,
            "bpk": np.ascontiguousarray(bpk),
            "bhw": np.ascontiguousarray(bhw),
        }
    else:
        wpk = swizzle([Wf.T @ Wg, wht, wot])
        common = {"wpk": wpk}

    return [
        {"x": np.ascontiguousarray(x[i].reshape(C, N)), **common}
        for i in range(NCORES)
    ]


def kernel(x, Wf, bf, Wg, bg, Wh, bh, Wo, bo):
    from concourse.bass_utils import run_bass_kernel_spmd

    inputs = {"x": x, "Wf": Wf, "bf": bf, "Wg": Wg, "bg": bg,
              "Wh": Wh, "bh": bh, "Wo": Wo, "bo": bo}
    use_bias = bool(
        np.any(np.asarray(bf)) or np.any(np.asarray(bg))
        or np.any(np.asarray(bh)) or np.any(np.asarray(bo))
    )
    nc = _get_graph(use_bias)
    in_maps = _make_in_maps(inputs, use_bias)
    res = None
    last_err = None
    for attempt in range(3):
        try:
            res = run_bass_kernel_spmd(nc, in_maps, list(range(NCORES)))
            break
        except Exception as e:  # transient device wedge (NRT unrecoverable)
            last_err = e
            if "UNRECOVERABLE" not in str(e) and "UNAVAILABLE" not in str(e):
                raise
            import time
            time.sleep(10)
    if res is None:
        raise last_err
    out = np.stack([res.results[i]["out"] for i in range(NCORES)])
    return np.ascontiguousarray(out.astype(np.float32).reshape(B, C, W_DIM, H_DIM))


# revision 38
# speedup vs baseline: 1.0910x; 1.0910x over previous
"""Trainium2 Bass kernel for nn_Attention_88493506167116.

Channel-attention module (per batch item b):
    F = (Wf @ X).reshape raw (N, C);  G likewise;  Hm likewise (X = x[b] as (C, N))
    S = G^T @ F  (C x C), beta = softmax(S, axis=-1)
    O = beta @ Hm^T  (C, N) -> reshape (C, W, H);  out = Wo @ O + bo

Key structure (C=256, N=4096 = 16*C): the raw reshape (C, N) -> (N, C) is a
block regrouping: F_r[16c+q, r] = Yf[c, q*C + r].  Hence with X_q = X[:, qC:(q+1)C]:
    S     = sum_q Yg_q^T Yf_q = sum_q X_q^T A X_q,   A = Wg^T Wf   (host-folded)
    out   = Wo @ O: with P = Wo @ beta, Out[o, 16c+q] = (P @ Yh_q^T)[o, c]
and Yh_q^T = YhT[qC:(q+1)C, :] where YhT = X^T Wh^T is computed directly in
transposed layout (no on-device transposes anywhere).

Sharding: pure data-parallel, batch B=8 across the 8 NeuronCores (one image
per core), zero collectives.  Host folds A = Wf.T @ Wg (lhsT layout) and
transposes weights.

The fast (zero-bias) path computes in float32r: fp32 storage/exponent, PE
rounds products to ~13 mantissa bits, 4x faster than fp32 matmul.  Measured
end-to-end error ~1.3e-3.  The general-bias path stays full fp32.

Hardware constraints honored: fp32/f32r matmuls self-load weights (S3_LW)
and, like DMA instructions, carry at most ONE sync wait.  So: weights packed
into one contiguous DMA; tiny fp32 warmup matmuls cover each DMA sem lane
before first use; PSUM pools are split so each pool's tiles are only ever
read by one engine (psW -> ACT, psV -> DVE), making every matmul's WAR +
input waits coalesce onto a single semaphore.  A post-pass splits any
residual multi-wait instruction (the tail drain) into single-wait no-ops.
"""

import numpy as np

B, C, W_DIM, H_DIM = 8, 256, 64, 64
N = W_DIM * H_DIM          # 4096
Q = N // C                 # 16
P = 128                    # partitions
NCORES = 8

_GRAPH_CACHE = {}


def _build_graph(use_bias: bool):
    from contextlib import ExitStack

    import concourse.bass as bass
    import concourse.tile as tile
    from concourse import mybir

    f32 = mybir.dt.float32
    f32r = mybir.dt.float32r
    AF = mybir.ActivationFunctionType

    nc = bass.Bass()

    dcomp = f32 if use_bias else f32r

    NW = 8 if use_bias else 6
    x_ext = nc.declare_dram_parameter("x", [C, N], dcomp, isOutput=False)
    # wpk is pre-swizzled on host to (P, NW*C): partition-major, contiguous
    # 6KB lines per partition -> fast DMA (vs 768 separate 1KB descriptors)
    wpk_ext = nc.declare_dram_parameter("wpk", [P, NW * C], dcomp, isOutput=False)
    if use_bias:
        bpk_ext = nc.declare_dram_parameter("bpk", [P, 6], f32, isOutput=False)
        bhw_ext = nc.declare_dram_parameter("bhw", [1, 3 * C], f32, isOutput=False)
    dout = f32 if use_bias else mybir.dt.bfloat16
    out_ext = nc.declare_dram_parameter("out", [C, N], dout, isOutput=True)

    with ExitStack() as ctx:
        tc = ctx.enter_context(tile.TileContext(nc))
        cpool = ctx.enter_context(tc.tile_pool(name="cpool", bufs=1))
        psS = ctx.enter_context(tc.tile_pool(name="psS", bufs=1, space="PSUM"))
        psW = ctx.enter_context(tc.tile_pool(name="psW", bufs=3, space="PSUM"))
        psV = ctx.enter_context(tc.tile_pool(name="psV", bufs=3, space="PSUM"))

        # pool-consistent evacuation engines: psW tiles are read only by the
        # scalar engine (ACT), psV tiles only by the vector engine (DVE)
        def evacA(dst, src):
            nc.scalar.copy(dst, src)

        def evacV(dst, src):
            nc.vector.tensor_copy(dst, src)

        if use_bias:
            evacV = evacA  # single engine keeps the wait discipline trivial

        def pick(i):
            """alternate (pool, evac) by index for load balance"""
            if use_bias:
                return psW, evacA
            return (psV, evacV) if i % 2 == 0 else (psW, evacA)

        # ---- resident SBUF tensors -------------------------------------
        # x loaded in column pieces per row-chunk (small first piece) so
        # compute can start as soon as ~0.5 MB lands.
        x_sb = [cpool.tile([P, N], dcomp, name=f"x{kc}") for kc in range(2)]
        wpk_sb = cpool.tile([P, NW, C], dcomp, name="wpk")
        nc.sync.dma_start(out=wpk_sb.rearrange("p a b -> p (a b)"), in_=wpk_ext[:])
        XCUTS = [0, 512, 1536, 2560, N]
        for h in range(len(XCUTS) - 1):
            for kc in range(2):
                nc.sync.dma_start(
                    out=x_sb[kc][:, XCUTS[h]:XCUTS[h + 1]],
                    in_=x_ext[kc * P:(kc + 1) * P, XCUTS[h]:XCUTS[h + 1]])

        if use_bias:
            wft_sb = [wpk_sb[:, 0 + kc, :] for kc in range(2)]
            wgt_sb = [wpk_sb[:, 2 + kc, :] for kc in range(2)]
            wht_sb = [wpk_sb[:, 4 + kc, :] for kc in range(2)]
            wot_sb = [wpk_sb[:, 6 + kc, :] for kc in range(2)]
            bpk_sb = cpool.tile([P, 6], f32, name="bpk")
            bhw_sb = cpool.tile([1, 3 * C], f32, name="bhw")
            nc.sync.dma_start(out=bpk_sb[:], in_=bpk_ext[:])
            nc.sync.dma_start(out=bhw_sb[:], in_=bhw_ext[:])
            bf_sb = [bpk_sb[:, 0 + kc:1 + kc] for kc in range(2)]
            bg_sb = [bpk_sb[:, 2 + kc:3 + kc] for kc in range(2)]
            bo_sb = [bpk_sb[:, 4 + kc:5 + kc] for kc in range(2)]
            bh2_row = bhw_sb[0:1, 0:2 * C]       # [bh | bh]
            wosum_row = bhw_sb[0:1, 2 * C:3 * C]
        else:
            at_sb = [wpk_sb[:, 0 + kc, :] for kc in range(2)]
            wht_sb = [wpk_sb[:, 2 + kc, :] for kc in range(2)]
            wot_sb = [wpk_sb[:, 4 + kc, :] for kc in range(2)]

        # ---- PE sync warmup (one tiny fp32 matmul per DMA sem lane) ----
        scratch_ps = psV.tile([P, 512], f32, name="v")

        def warmup(t):
            nc.tensor.matmul(scratch_ps[:, 0:1], t.bitcast(f32),
                             t[:, 0:1].bitcast(f32), start=True, stop=True)

        if not use_bias:
            # HAM pre-warm: keep PE busy on dummy fp32 matmuls while the x /
            # weight DMAs land, so real matmuls start at 2.4 GHz not 1.2.
            # Emitted before the lane warmups (those block on the DMAs).
            # fp32 matmuls lower to two half-speed passes, so each dummy is
            # ~0.4-0.9us; 8 of them spans the DMA head: HAM flips to 8/8
            # after ~3.4us of PE busy and stays warm into the real work
            ham_ct = cpool.tile([P, 256], f32, name="ham_ct")
            nc.vector.memset(ham_ct[:], 0.7182818)
            for _ in range(8):
                nc.tensor.matmul(scratch_ps[:, 0:256], ham_ct[:, 0:P],
                                 ham_ct[:], start=True, stop=True)

        warmup(wpk_sb[:, 0, 0:P])
        warmup(x_sb[0][:, 0:P])
        warmup(x_sb[1][:, 0:P])
        if use_bias:
            for h in range(1, 4):
                warmup(x_sb[0][:, XCUTS[h]:XCUTS[h] + P])
                warmup(x_sb[1][:, XCUTS[h]:XCUTS[h] + P])
            nc.tensor.matmul(scratch_ps[0:1, 0:1], bhw_sb[0:1, 0:1],
                             bhw_sb[0:1, 0:1], start=True, stop=True)
            act_scr = cpool.tile([P, 1], f32, name="act_scr")
            nc.scalar.copy(act_scr[:], bpk_sb[:, 0:1])

        # S accumulator PSUM tiles, pinned across the whole contraction
        psS_t = [psS.tile([P, C], f32, name=f"S{ac}") for ac in range(2)]

        # ================================================================
        # Pre-softmax path: fill psS_t[ac] with S = G_r^T F_r
        # ================================================================
        if not use_bias:
            # T_q = A @ X_q (two q at a time, 512-wide), then S += X_q^T T_q
            t2_sb = [[cpool.tile([P, 2 * C], dcomp, name=f"t{qp}_{uc}")
                      for uc in range(2)] for qp in range(Q // 2)]
            for qp in range(Q // 2):
                if qp in (1, 3, 5):  # next x column piece arrives
                    h = (qp + 1) // 2
                    warmup(x_sb[0][:, XCUTS[h]:XCUTS[h] + P])
                    warmup(x_sb[1][:, XCUTS[h]:XCUTS[h] + P])
                for uc in range(2):
                    pool, ev = pick(qp * 2 + uc)
                    ps = pool.tile([P, 2 * C], f32,
                                   name="v" if pool is psV else "w")
                    for kc in range(2):
                        nc.tensor.matmul(
                            ps[:],
                            at_sb[kc][:, uc * P:(uc + 1) * P],
                            x_sb[kc][:, qp * 2 * C:(qp + 1) * 2 * C],
                            start=(kc == 0), stop=(kc == 1),
                        )
                    ev(t2_sb[qp][uc][:], ps[:])
            for ac in range(2):
                for q in range(Q):
                    for uc in range(2):
                        nc.tensor.matmul(
                            psS_t[ac][:],
                            x_sb[uc][:, q * C + ac * P: q * C + ac * P + P],
                            t2_sb[q // 2][uc][:, (q % 2) * C:(q % 2 + 1) * C],
                            start=(q == 0 and uc == 0),
                            stop=(q == Q - 1 and uc == 1),
                        )
        else:
            # materialize Yf = Wf X + bf and Yg = Wg X + bg, then
            # S = sum_q Yg_q^T Yf_q
            yf_sb = [cpool.tile([P, N], f32, name=f"yf{mc}") for mc in range(2)]
            yg_sb = [cpool.tile([P, N], f32, name=f"yg{mc}") for mc in range(2)]
            for mc in range(2):
                for nb in range(8):
                    nsl = slice(nb * 512, (nb + 1) * 512)
                    ps = psW.tile([P, 512], f32, name="w")
                    for kc in range(2):
                        nc.tensor.matmul(
                            ps[:], wft_sb[kc][:, mc * P:(mc + 1) * P],
                            x_sb[kc][:, nsl], start=(kc == 0), stop=(kc == 1))
                    nc.scalar.activation(yf_sb[mc][:, nsl], ps[:], AF.Identity,
                                         bias=bf_sb[mc], scale=1.0)
                    ps = psW.tile([P, 512], f32, name="w")
                    for kc in range(2):
                        nc.tensor.matmul(
                            ps[:], wgt_sb[kc][:, mc * P:(mc + 1) * P],
                            x_sb[kc][:, nsl], start=(kc == 0), stop=(kc == 1))
                    nc.scalar.activation(yg_sb[mc][:, nsl], ps[:], AF.Identity,
                                         bias=bg_sb[mc], scale=1.0)
            for ac in range(2):
                for q in range(Q):
                    for kc in range(2):
                        nc.tensor.matmul(
                            psS_t[ac][:],
                            yg_sb[kc][:, q * C + ac * P: q * C + ac * P + P],
                            yf_sb[kc][:, q * C:(q + 1) * C],
                            start=(q == 0 and kc == 0),
                            stop=(q == Q - 1 and kc == 1),
                        )

        # ================================================================
        # YhT = X^T @ Wh^T in (N, C) layout: 8 quad tiles (128, 4C), quad u
        # holds row-chunks 4u..4u+3 at column offsets j*C
        # ================================================================
        yht_q4 = [cpool.tile([P, 4 * C], dcomp, name=f"yht{u}")
                  for u in range(Q // 2)]
        for u in range(Q // 2):
            for g in range(2):
                pool, ev = pick(2 * u + g)
                ps = pool.tile([P, 2 * C], f32, name="v" if pool is psV else "w")
                for half in range(2):
                    nch = 4 * u + 2 * g + half
                    for kc in range(2):
                        nc.tensor.matmul(
                            ps[:, half * C:(half + 1) * C],
                            x_sb[kc][:, nch * P:(nch + 1) * P],
                            wht_sb[kc][:],
                            start=(kc == 0), stop=(kc == 1),
                        )
                ev(yht_q4[u][:, g * 2 * C:(g + 1) * 2 * C], ps[:])

        # ================================================================
        # softmax rows of S -> beta (normalized), in SBUF
        # ================================================================
        beta_sb = [cpool.tile([P, C], dcomp, name=f"beta{ac}") for ac in range(2)]
        for ac in range(2):
            negmax = cpool.tile([P, 1], f32, name=f"negmax{ac}")
            sumexp = cpool.tile([P, 1], f32, name=f"sumexp{ac}")
            rcp = cpool.tile([P, 1], f32, name=f"rcp{ac}")
            expo = cpool.tile([P, C], f32, name=f"expo{ac}")
            nc.vector.tensor_reduce(
                out=negmax[:], in_=psS_t[ac][:],
                axis=mybir.AxisListType.X, op=mybir.AluOpType.max, negate=True)
            nc.scalar.activation(
                expo[:], psS_t[ac][:], AF.Exp,
                bias=negmax[:, 0:1], scale=1.0, accum_out=sumexp[:, 0:1])
            nc.vector.reciprocal(rcp[:], sumexp[:])
            if use_bias:
                nc.scalar.activation(beta_sb[ac][:], expo[:], AF.Copy,
                                     bias=0.0, scale=rcp[:, 0:1])
            else:
                nc.vector.tensor_scalar_mul(beta_sb[ac][:], expo[:], rcp[:, 0:1])

        # ================================================================
        # P^T = beta^T @ Wo^T   (2 tiles (128, C), j' on partitions)
        # ================================================================
        pt_sb = [cpool.tile([P, C], dcomp, name=f"pt{j}") for j in range(2)]
        for jpc in range(2):
            pool = psW if use_bias else psV
            ps = pool.tile([P, 2 * C], f32, name="w" if use_bias else "v")
            for jc in range(2):
                nc.tensor.matmul(
                    ps[:, 0:C],
                    beta_sb[jc][:, jpc * P:(jpc + 1) * P],
                    wot_sb[jc][:],
                    start=(jc == 0), stop=(jc == 1),
                )
            (evacA if use_bias else evacV)(pt_sb[jpc][:], ps[:, 0:C])

        # ================================================================
        # Out[o, 16c+q] = (P @ Yh_q^T)[o, c] (+ wosum[o]*bh[c] + bo[o])
        # Two q per PSUM tile; one paired (transposing-AP) evacuation.
        # ================================================================
        for oc in range(2):
            out_sb = cpool.tile([P, C, Q], dout, name=f"out{oc}")
            for u in range(Q // 2):
                pool, ev = pick(u + oc)
                ps = pool.tile([P, 2 * C], f32, name="v" if pool is psV else "w")
                # rhs covers q=2u (cols 0:C) and q=2u+1 (C:2C) in one 512-wide
                # strided stream: chunks {4u+jc, 4u+2+jc} of YhT
                rhsv = yht_q4[u].rearrange("p (x y c) -> p y x c", x=2, y=2)
                for jc in range(2):
                    nc.tensor.matmul(
                        ps[:],
                        pt_sb[jc][:, oc * P:(oc + 1) * P],
                        rhsv[:, jc],
                        start=(jc == 0),
                        stop=(jc == 1 and not use_bias),
                    )
                if use_bias:
                    nc.tensor.matmul(
                        ps[:],
                        wosum_row[:, oc * P:(oc + 1) * P],
                        bh2_row[:],
                        start=False, stop=True,
                    )
                    nc.scalar.activation(
                        out_sb[:, :, 2 * u:2 * u + 2],
                        ps.rearrange("p (h c) -> p c h", h=2),
                        AF.Identity, bias=bo_sb[oc], scale=1.0)
                else:
                    ev(out_sb[:, :, 2 * u:2 * u + 2],
                       ps.rearrange("p (h c) -> p c h", h=2))
            nc.sync.dma_start(
                out=out_ext[oc * P:(oc + 1) * P, :],
                in_=out_sb.rearrange("p c q -> p (c q)"),
            )

    return nc


def _split_multiwait_insts(nc, max_waits: int = 1):
    """walrus rejects instructions carrying more than one sync wait; hoist
    extra waits onto same-engine no-ops placed immediately before."""
    from concourse import mybir

    nop_id = 0
    for fn in nc.m.functions:
        for blk in fn.blocks:
            insts = list(blk.instructions)
            new_list = []
            changed = False
            for inst in insts:
                si = inst.sync_info
                if si is not None and len(si.on_wait) > max_waits:
                    waits = list(si.on_wait)
                    for w in waits[:-max_waits]:
                        nop = mybir.InstNoOp(name=f"I-waitnop{nop_id}", ins=[],
                                             outs=[])
                        nop_id += 1
                        nop.engine = inst.engine
                        nop.sync_info = mybir.SyncInfo(on_wait=[w], on_update=[])
                        new_list.append(nop)
                    inst.sync_info = mybir.SyncInfo(
                        on_wait=waits[-max_waits:],
                        on_update=list(si.on_update),
                    )
                    changed = True
                new_list.append(inst)
            if changed:
                blk.instructions = new_list
    return nc


def _get_graph(use_bias: bool):
    key = bool(use_bias)
    if key not in _GRAPH_CACHE:
        _GRAPH_CACHE[key] = _split_multiwait_insts(_build_graph(key))
    return _GRAPH_CACHE[key]


def _make_in_maps(inputs, use_bias):
    x = np.ascontiguousarray(np.asarray(inputs["x"], dtype=np.float32))
    Wf = np.asarray(inputs["Wf"], dtype=np.float32)
    Wg = np.asarray(inputs["Wg"], dtype=np.float32)
    Wh = np.asarray(inputs["Wh"], dtype=np.float32)
    Wo = np.asarray(inputs["Wo"], dtype=np.float32)

    wht = np.ascontiguousarray(Wh.T)
    wot = np.ascontiguousarray(Wo.T)

    def swizzle(wlist):
        # stack (NW, 128, C) row-chunks then move partitions outermost:
        # wpk[p, g*C:(g+1)*C] = chunk g row p  ->  shape (P, NW*C)
        chunks = []
        for w in wlist:
            chunks.append(w[:P])
            chunks.append(w[P:])
        arr = np.stack(chunks, axis=0)           # (NW, P, C)
        return np.ascontiguousarray(
            arr.transpose(1, 0, 2).reshape(P, -1))

    if use_bias:
        bf = np.asarray(inputs["bf"], np.float32)
        bg = np.asarray(inputs["bg"], np.float32)
        bh = np.asarray(inputs["bh"], np.float32)
        bo = np.asarray(inputs["bo"], np.float32)
        wpk = swizzle([Wf.T, Wg.T, wht, wot])
        bpk = np.stack([bf[:P], bf[P:], bg[:P], bg[P:], bo[:P], bo[P:]], axis=1)
        bhw = np.concatenate([bh, bh, Wo.sum(axis=1)]).reshape(1, 3 * C)
        common = {
            "wpk": wpk,
            "bpk": np.ascontiguousarray(bpk),
            "bhw": np.ascontiguousarray(bhw),
        }
    else:
        wpk = swizzle([Wf.T @ Wg, wht, wot])
        common = {"wpk": wpk}

    return [
        {"x": np.ascontiguousarray(x[i].reshape(C, N)), **common}
        for i in range(NCORES)
    ]


def kernel(x, Wf, bf, Wg, bg, Wh, bh, Wo, bo):
    from concourse.bass_utils import run_bass_kernel_spmd

    inputs = {"x": x, "Wf": Wf, "bf": bf, "Wg": Wg, "bg": bg,
              "Wh": Wh, "bh": bh, "Wo": Wo, "bo": bo}
    use_bias = bool(
        np.any(np.asarray(bf)) or np.any(np.asarray(bg))
        or np.any(np.asarray(bh)) or np.any(np.asarray(bo))
    )
    nc = _get_graph(use_bias)
    in_maps = _make_in_maps(inputs, use_bias)
    res = None
    last_err = None
    for attempt in range(3):
        try:
            res = run_bass_kernel_spmd(nc, in_maps, list(range(NCORES)))
            break
        except Exception as e:  # transient device wedge (NRT unrecoverable)
            last_err = e
            if "UNRECOVERABLE" not in str(e) and "UNAVAILABLE" not in str(e):
                raise
            import time
            time.sleep(10)
    if res is None:
        raise last_err
    out = np.stack([res.results[i]["out"] for i in range(NCORES)])
    return np.ascontiguousarray(out.astype(np.float32).reshape(B, C, W_DIM, H_DIM))
